# revision 22
# baseline (speedup 1.0000x reference)
import sys

if "/opt/trn_rl_repo" not in sys.path:
    sys.path.insert(0, "/opt/trn_rl_repo")

import numpy as np

from concourse import bacc, bass, mybir, tile

F32 = mybir.dt.float32
F32R = mybir.dt.float32r
BF16 = mybir.dt.bfloat16

B, T, C = 4, 2048, 1024
H, D = 16, 64
HL = 8          # local heads per core (head-group of 8)
SCALE = float(C) ** -0.5  # 1/32


def _view3(ap2, d1, d2):
    """View a contiguous [P, d1*d2] AP as [P, d1, d2]."""
    return bass.AP(
        tensor=ap2.tensor,
        offset=ap2.offset,
        ap=[ap2.ap[0], [d2, d1], [1, d2]],
    )


def _bcast_rows(ap_rows, reps, free):
    """[2, free] AP -> [2*reps, free] AP replicating each row `reps` times."""
    return bass.AP(
        tensor=ap_rows.tensor,
        offset=ap_rows.offset,
        ap=[[ap_rows.ap[0][0], 2], [0, reps], [1, free]],
    )


def _emit(nc, tc, xT_d, wq_d, wk_d, wv_d, wp_d, y_d):
    r = F32R

    with tc.tile_pool(name="persist", bufs=1) as pp:
        QT = [pp.tile([128, T], BF16, name=f"qt{p}") for p in range(4)]
        KT = [pp.tile([128, T], BF16, name=f"kt{p}") for p in range(4)]
        VP = [pp.tile([128, HL, D + 1], BF16, name=f"vp{s}") for s in range(16)]

        for st in range(16):
            nc.gpsimd.memset(VP[st][:, :, D : D + 1], 1.0)

        # ---------------- Phase A: QKV projections ----------------
        with tc.tile_pool(name="xw", bufs=1) as xw, tc.tile_pool(
            name="paps", bufs=1, space="PSUM"
        ) as paps:
            xts = [xw.tile([128, T], BF16, name=f"xt{ct}") for ct in range(8)]
            for ct in range(8):
                nc.sync.dma_start(xts[ct][:], xT_d[ct * 128 : (ct + 1) * 128, :])

            def load_w(wd, tag):
                wt = [xw.tile([128, 512], BF16, name=f"w{tag}{ct}") for ct in range(8)]
                for ct in range(8):
                    nc.sync.dma_start(wt[ct][:], wd[ct * 128 : (ct + 1) * 128, :])
                return wt

            def qk_wave(wt, out_tiles, tbs):
                ps = {}
                for p in range(4):
                    for tb in tbs:
                        ps[(p, tb)] = paps.tile([128, 512], F32, name=f"ps{p}_{tb % 2}")
                for ct in range(8):
                    for p in range(4):
                        for tb in tbs:
                            nc.tensor.matmul(
                                ps[(p, tb)][:],
                                wt[ct][:, p * 128 : (p + 1) * 128],
                                xts[ct][:, tb * 512 : (tb + 1) * 512],
                                start=(ct == 0),
                                stop=(ct == 7),
                            )
                for p in range(4):
                    for tb in tbs:
                        nc.vector.tensor_copy(
                            out_tiles[p][:, tb * 512 : (tb + 1) * 512], ps[(p, tb)][:]
                        )

            def v_wave(wt, sts):
                ps = {}
                for st in sts:
                    ps[st] = paps.tile([128, 512], F32, name=f"ps{(st % 8) // 2}_{st % 2}")
                for ct in range(8):
                    for st in sts:
                        nc.tensor.matmul(
                            ps[st][:],
                            xts[ct][:, st * 128 : (st + 1) * 128],
                            wt[ct][:],
                            start=(ct == 0),
                            stop=(ct == 7),
                        )
                for st in sts:
                    nc.vector.tensor_copy(
                        VP[st][:, :, 0:D], _view3(ps[st][:], HL, D)
                    )

            wqt = load_w(wq_d, "q")
            wkt = load_w(wk_d, "k")
            wvt = load_w(wv_d, "v")
            qk_wave(wqt, QT, (0, 1))
            qk_wave(wqt, QT, (2, 3))
            qk_wave(wkt, KT, (0, 1))
            qk_wave(wkt, KT, (2, 3))
            v_wave(wvt, range(0, 8))
            v_wave(wvt, range(8, 16))

        # ---------------- Attention ----------------
        with tc.tile_pool(name="otp", bufs=1) as otp:
            OT = [otp.tile([128, T], BF16, name=f"ot{p}") for p in range(4)]

            with tc.tile_pool(name="wproj", bufs=1) as wpp:
                wpt = [wpp.tile([128, 1024], BF16, name=f"wp{p}") for p in range(4)]
                for p in range(4):
                    nc.sync.dma_start(wpt[p][:], wp_d[p * 128 : (p + 1) * 128, :])

                with tc.tile_pool(name="attn", bufs=1) as atp, tc.tile_pool(
                    name="expp", bufs=2
                ) as expp, tc.tile_pool(name="stgp", bufs=3) as stgp, tc.tile_pool(
                    name="scps", bufs=1, space="PSUM"
                ) as scps, tc.tile_pool(
                    name="avps", bufs=1, space="PSUM"
                ) as avps:
                    ones = atp.tile([128, 512], BF16, name="ones")
                    nc.vector.memset(ones[:], 1.0)
                    masks = [atp.tile([128, 512], BF16, name=f"mask{m}") for m in range(4)]
                    for m in range(4):
                        # keep where f >= 128*m + partition  (causal diag block)
                        nc.gpsimd.affine_select(
                            masks[m][:],
                            ones[:],
                            [[1, 512]],
                            mybir.AluOpType.is_ge,
                            0.0,
                            base=-128 * m,
                            channel_multiplier=-1,
                        )
                    denom = atp.tile([128, T], F32R, name="denom")

                    ALLCH = []
                    for lh in range(HL):
                        for st in range(16):
                            tbs = list(range(st // 4, 4))
                            for c0 in range(0, len(tbs), 2):
                                ALLCH.append((lh, st, tbs[c0 : c0 + 2], c0 == 0))

                    sc_t = {}

                    def issue_scores(i):
                        lh, st, tbs, _ = ALLCH[i]
                        p, half = lh // 2, lh % 2
                        hs = slice(half * 64, (half + 1) * 64)
                        sct = scps.tile([128, 1024], F32, name=f"sc{i % 2}")
                        sc_t[i] = sct
                        for j, tb in enumerate(tbs):
                            nc.tensor.matmul(
                                sct[:, j * 512 : (j + 1) * 512],
                                KT[p][hs, st * 128 : (st + 1) * 128],
                                QT[p][hs, tb * 512 : (tb + 1) * 512],
                                start=True,
                                stop=True,
                            )

                    av = None
                    issue_scores(0)
                    for i, (lh, st, tbs, first) in enumerate(ALLCH):
                        p, half = lh // 2, lh % 2
                        hs = slice(half * 64, (half + 1) * 64)
                        if st == 0 and first:
                            av = [
                                avps.tile([128, 512], F32, name=f"av{tb}")
                                for tb in range(4)
                            ]
                        w = len(tbs) * 512
                        ex = expp.tile([128, 1024], BF16, name=f"ex{i % 2}")
                        nc.scalar.activation(
                            ex[:, 0:w],
                            sc_t.pop(i)[:, 0:w],
                            mybir.ActivationFunctionType.Exp,
                            scale=SCALE,
                        )
                        if i + 1 < len(ALLCH):
                            issue_scores(i + 1)
                        if first:
                            nc.vector.tensor_mul(
                                ex[:, 0:512], ex[:, 0:512], masks[st % 4][:]
                            )
                        for j, tb in enumerate(tbs):
                            nc.tensor.matmul(
                                av[tb][0 : D + 1, :],
                                VP[st][:, lh, :],
                                ex[:, j * 512 : (j + 1) * 512],
                                start=(st == 0),
                                stop=(st == 4 * tb + 3),
                            )
                            if st == 4 * tb + 3:
                                stg = stgp.tile([128, 512], BF16, name="stg")
                                nc.vector.tensor_copy(stg[0:D, :], av[tb][0:D, :])
                                nc.sync.dma_start(
                                    OT[p][hs, tb * 512 : (tb + 1) * 512],
                                    stg[0:D, :],
                                )
                                stgd = stgp.tile([128, 512], F32R, name="stgd")
                                nc.vector.tensor_copy(
                                    stgd[D : D + 1, :], av[tb][D : D + 1, :]
                                )
                                nc.sync.dma_start(
                                    denom[lh : lh + 1, tb * 512 : (tb + 1) * 512],
                                    stgd[D : D + 1, :],
                                )

                    # normalize: OT[p] *= 1/denom broadcast over 64 partitions/head
                    nc.vector.reciprocal(
                        denom[0:HL, :].bitcast(F32), denom[0:HL, :].bitcast(F32)
                    )
                    with tc.tile_pool(name="rcpp", bufs=1) as rcpp:
                        for p in range(4):
                            rcp = rcpp.tile([128, T], F32R, name="rcp")
                            nc.sync.dma_start(
                                rcp[:], _bcast_rows(denom[2 * p : 2 * p + 2, :], 64, T)
                            )
                            nc.vector.tensor_mul(OT[p][:], OT[p][:], rcp[:])

                # ---------------- Output projection ----------------
                with tc.tile_pool(name="ysb", bufs=4) as ysb, tc.tile_pool(
                    name="pps", bufs=4, space="PSUM"
                ) as pps:
                    for ts in range(16):
                        for cb in range(2):
                            ps = pps.tile([128, 512], F32, name="yp")
                            for p in range(4):
                                nc.tensor.matmul(
                                    ps[:],
                                    OT[p][:, ts * 128 : (ts + 1) * 128],
                                    wpt[p][:, cb * 512 : (cb + 1) * 512],
                                    start=(p == 0),
                                    stop=(p == 3),
                                )
                            yt = ysb.tile([128, 512], F32, name="ys")
                            nc.vector.tensor_copy(yt[:], ps[:])
                            nc.sync.dma_start(
                                y_d[ts * 128 : (ts + 1) * 128, cb * 512 : (cb + 1) * 512],
                                yt[:],
                            )


def _build():
    nc = bacc.Bacc("TRN2", target_bir_lowering=False, debug=False)
    xT_d = nc.dram_tensor("xT", [C, T], BF16, kind="ExternalInput")
    wq_d = nc.dram_tensor("wq", [C, 512], BF16, kind="ExternalInput")
    wk_d = nc.dram_tensor("wk", [C, 512], BF16, kind="ExternalInput")
    wv_d = nc.dram_tensor("wv", [C, 512], BF16, kind="ExternalInput")
    wp_d = nc.dram_tensor("wp", [512, C], F32R, kind="ExternalInput")
    y_d = nc.dram_tensor("y", [T, C], F32, kind="ExternalOutput")
    with tile.TileContext(nc) as tc:
        _emit(nc, tc, xT_d, wq_d, wk_d, wv_d, wp_d, y_d)
    nc.compile()
    return nc


_PROG = None
_RUN = None
LAST_EXEC_NS = None
TRACE = False


def _get_prog():
    global _PROG
    if _PROG is None:
        _PROG = _build()
    return _PROG


def _get_run(nc):
    # run_bass_via_pjrt builds a fresh jax.jit closure (and re-serializes the
    # BIR at lowering) on every call; cache the jitted executable instead.
    global _RUN
    if _RUN is not None:
        return _RUN
    import jax
    from concourse.bass2jax import (
        Mesh,
        PartitionSpec,
        _bass_exec_p,
        install_neuronx_cc_hook,
        partition_id_tensor,
        shard_map,
    )

    install_neuronx_cc_hook()
    assert nc.dbg_addr is None

    partition_name = nc.partition_id_tensor.name if nc.partition_id_tensor else None
    in_names = []
    out_names = []
    out_avals = []
    out_shapes = []
    for alloc in nc.m.functions[0].allocations:
        if not isinstance(alloc, mybir.MemoryLocationSet):
            continue
        name = alloc.memorylocations[0].name
        if alloc.kind == "ExternalInput":
            if name != partition_name:
                in_names.append(name)
        elif alloc.kind == "ExternalOutput":
            out_names.append(name)
            shape = tuple(alloc.tensor_shape)
            dtype = mybir.dt.np(alloc.dtype)
            out_avals.append(jax.core.ShapedArray(shape, dtype))
            out_shapes.append((shape, dtype))
    n_params = len(in_names)
    n_outs = len(out_names)
    all_names = in_names + out_names + ([partition_name] if partition_name else [])
    donate = tuple(range(n_params, n_params + n_outs))

    def _body(*args):
        operands = list(args)
        if partition_name is not None:
            operands.append(partition_id_tensor())
        outs = _bass_exec_p.bind(
            *operands,
            out_avals=tuple(out_avals),
            in_names=tuple(all_names),
            out_names=tuple(out_names),
            lowering_input_output_aliases=(),
            sim_require_finite=True,
            sim_require_nnan=True,
            nc=nc,
        )
        return tuple(outs)

    devices = jax.devices()[:8]
    mesh = Mesh(np.asarray(devices), ("core",))
    in_specs = (PartitionSpec("core"),) * (n_params + n_outs)
    out_specs = (PartitionSpec("core"),) * n_outs
    sharded = jax.jit(
        shard_map(
            _body, mesh=mesh, in_specs=in_specs, out_specs=out_specs, check_rep=False
        ),
        donate_argnums=donate,
        keep_unused=True,
    )

    def run(in_maps):
        concat_in = [
            np.concatenate([m[name] for m in in_maps], axis=0) for name in in_names
        ]
        concat_zeros = [
            np.zeros((8 * shape[0], *shape[1:]), dt) for (shape, dt) in out_shapes
        ]
        out_arrs = sharded(*concat_in, *concat_zeros)
        return [
            {
                name: np.asarray(out_arrs[i]).reshape(8, *out_shapes[i][0])[c]
                for i, name in enumerate(out_names)
            }
            for c in range(8)
        ]

    _RUN = run
    return run


def kernel(x, Wq, Wk, Wv, Wp, bp):
    global LAST_EXEC_NS
    import ml_dtypes

    BF = ml_dtypes.bfloat16
    nc = _get_prog()
    x = np.asarray(x, np.float32)
    in_maps = []
    for core in range(8):
        b, g = divmod(core, 2)
        hsl = slice(g * 8, (g + 1) * 8)
        in_maps.append(
            {
                "xT": np.ascontiguousarray(x[b].T).astype(BF),
                "wq": np.ascontiguousarray(
                    np.asarray(Wq, np.float32)[hsl].transpose(1, 0, 2).reshape(C, 512)
                ).astype(BF),
                "wk": np.ascontiguousarray(
                    np.asarray(Wk, np.float32)[hsl].transpose(1, 0, 2).reshape(C, 512)
                ).astype(BF),
                "wv": np.ascontiguousarray(
                    np.asarray(Wv, np.float32)[hsl].transpose(1, 0, 2).reshape(C, 512)
                ).astype(BF),
                "wp": np.ascontiguousarray(np.asarray(Wp, np.float32)[g * 512 : (g + 1) * 512, :]),
            }
        )
    outs = _get_run(nc)(in_maps)
    y = np.empty((B, T, C), np.float32)
    bpf = np.asarray(bp, np.float32)
    for b in range(B):
        y[b] = outs[2 * b]["y"] + outs[2 * b + 1]["y"] + bpf[None, :]
    return y


# revision 31
# speedup vs baseline: 1.1393x; 1.1393x over previous
import sys

if "/opt/trn_rl_repo" not in sys.path:
    sys.path.insert(0, "/opt/trn_rl_repo")

import numpy as np

from concourse import bacc, bass, mybir, tile

F32 = mybir.dt.float32
F32R = mybir.dt.float32r
BF16 = mybir.dt.bfloat16

B, T, C = 4, 2048, 1024
H, D = 16, 64
HL = 8          # local heads per core (head-group of 8)
SCALE = float(C) ** -0.5  # 1/32


def _view3(ap2, d1, d2):
    """View a contiguous [P, d1*d2] AP as [P, d1, d2]."""
    return bass.AP(
        tensor=ap2.tensor,
        offset=ap2.offset,
        ap=[ap2.ap[0], [d2, d1], [1, d2]],
    )


def _bcast_rows(ap_rows, reps, free):
    """[2, free] AP -> [2*reps, free] AP replicating each row `reps` times."""
    return bass.AP(
        tensor=ap_rows.tensor,
        offset=ap_rows.offset,
        ap=[[ap_rows.ap[0][0], 2], [0, reps], [1, free]],
    )


def _attn_schedule():
    """Chunk list (two passes) + filler placement (latest-fit before deadlines)."""
    chunks = []
    for lh in range(HL):  # pass A: tb 0/1 accumulators
        for st in range(8):
            chunks.append((lh, st, [0, 1] if st < 4 else [1]))
    for lh in range(HL):  # pass B: tb 2/3 accumulators
        for st in range(16):
            chunks.append((lh, st, [2, 3] if st < 12 else [3]))

    first_score = {}
    first_av = {}
    for c, (lh, st, tbs) in enumerate(chunks):
        p = lh // 2
        first_score.setdefault(("K", p, st // 4), c)
        for tb in tbs:
            first_score.setdefault(("Q", p, tb), c)
        first_av.setdefault(("V", st, 0), c)

    deadline = {}
    for fid, c in first_score.items():
        deadline[fid] = c - 2  # filler(c) emitted before scores(c+2)
    for fid, c in first_av.items():
        deadline[fid] = c  # V filler(c) emitted before AV(c)

    occupied = set()
    slot_of = {}
    pre = []
    for fid in sorted(deadline, key=lambda f: -deadline[f]):
        s = deadline[fid]
        while s >= 0 and s in occupied:
            s -= 1
        if s < 0:
            pre.append(fid)
        else:
            occupied.add(s)
            slot_of.setdefault(s, []).append(fid)
    pre.reverse()
    return chunks, pre, slot_of


def _emit(nc, tc, xT_d, wq_d, wk_d, wv_d, wp_d, y_d):
    with tc.tile_pool(name="persist", bufs=1) as pp:
        QT = [pp.tile([128, T], BF16, name=f"qt{p}") for p in range(4)]
        KT = [pp.tile([128, T], BF16, name=f"kt{p}") for p in range(4)]
        VP = [pp.tile([128, HL, D + 1], BF16, name=f"vp{s}") for s in range(16)]
        OT = [pp.tile([128, T], BF16, name=f"ot{p}") for p in range(4)]
        wpt = [pp.tile([128, 1024], BF16, name=f"wp{p}") for p in range(4)]

        for st in range(16):
            nc.gpsimd.memset(VP[st][:, :, D : D + 1], 1.0)

        with tc.tile_pool(name="xw", bufs=1) as xw:
            xts = [xw.tile([128, T], BF16, name=f"xt{ct}") for ct in range(8)]
            for ct in range(8):
                nc.sync.dma_start(xts[ct][:], xT_d[ct * 128 : (ct + 1) * 128, :])

            def load_w(wd, tag):
                wt = [xw.tile([128, 512], BF16, name=f"w{tag}{ct}") for ct in range(8)]
                for ct in range(8):
                    nc.sync.dma_start(wt[ct][:], wd[ct * 128 : (ct + 1) * 128, :])
                return wt

            wkt = load_w(wk_d, "k")
            wqt = load_w(wq_d, "q")
            wvt = load_w(wv_d, "v")
            for p in range(4):
                nc.sync.dma_start(wpt[p][:], wp_d[p * 128 : (p + 1) * 128, :])

            with tc.tile_pool(name="attn", bufs=1) as atp, tc.tile_pool(
                name="expp", bufs=2
            ) as expp, tc.tile_pool(name="stgp", bufs=3) as stgp, tc.tile_pool(
                name="scps", bufs=1, space="PSUM"
            ) as scps, tc.tile_pool(
                name="avps", bufs=1, space="PSUM"
            ) as avps, tc.tile_pool(
                name="fpp", bufs=2, space="PSUM"
            ) as fpp:
                ones = atp.tile([128, 512], BF16, name="ones")
                nc.vector.memset(ones[:], 1.0)
                masks = [atp.tile([128, 512], BF16, name=f"mask{m}") for m in range(4)]
                for m in range(4):
                    # keep where f >= 128*m + partition  (causal diag block)
                    nc.gpsimd.affine_select(
                        masks[m][:],
                        ones[:],
                        [[1, 512]],
                        mybir.AluOpType.is_ge,
                        0.0,
                        base=-128 * m,
                        channel_multiplier=-1,
                    )
                denom = atp.tile([128, T], F32R, name="denom")

                CHUNKS, PRE, SLOT = _attn_schedule()

                def emit_filler(fid):
                    kind = fid[0]
                    fp = fpp.tile([128, 512], F32, name="fps")
                    if kind == "V":
                        st = fid[1]
                        for ct in range(8):
                            nc.tensor.matmul(
                                fp[:],
                                xts[ct][:, st * 128 : (st + 1) * 128],
                                wvt[ct][:],
                                start=(ct == 0),
                                stop=(ct == 7),
                            )
                        nc.vector.tensor_copy(VP[st][:, :, 0:D], _view3(fp[:], HL, D))
                    else:
                        _, p, tb = fid
                        wt, dst = (wqt, QT) if kind == "Q" else (wkt, KT)
                        for ct in range(8):
                            nc.tensor.matmul(
                                fp[:],
                                wt[ct][:, p * 128 : (p + 1) * 128],
                                xts[ct][:, tb * 512 : (tb + 1) * 512],
                                start=(ct == 0),
                                stop=(ct == 7),
                            )
                        nc.vector.tensor_copy(dst[p][:, tb * 512 : (tb + 1) * 512], fp[:])

                sc_t = {}

                def issue_scores(i):
                    lh, st, tbs = CHUNKS[i]
                    p, half = lh // 2, lh % 2
                    hs = slice(half * 64, (half + 1) * 64)
                    sct = scps.tile([128, 1024], F32, name=f"sc{i % 2}")
                    sc_t[i] = sct
                    for j, tb in enumerate(tbs):
                        nc.tensor.matmul(
                            sct[:, j * 512 : (j + 1) * 512],
                            KT[p][hs, st * 128 : (st + 1) * 128],
                            QT[p][hs, tb * 512 : (tb + 1) * 512],
                            start=True,
                            stop=True,
                        )

                for fid in PRE:
                    emit_filler(fid)
                issue_scores(0)

                av = None
                for i, (lh, st, tbs) in enumerate(CHUNKS):
                    p, half = lh // 2, lh % 2
                    hs = slice(half * 64, (half + 1) * 64)
                    if st == 0:
                        av = {
                            tbs[0]: avps.tile([128, 512], F32, name="avA"),
                            tbs[1]: avps.tile([128, 512], F32, name="avB"),
                        }
                    diag = st // 4 == tbs[0]
                    w = len(tbs) * 512
                    ex = expp.tile([128, 1024], BF16, name=f"ex{i % 2}")
                    nc.scalar.activation(
                        ex[:, 0:w],
                        sc_t.pop(i)[:, 0:w],
                        mybir.ActivationFunctionType.Exp,
                        scale=SCALE,
                    )
                    if i + 1 < len(CHUNKS):
                        issue_scores(i + 1)
                    here = SLOT.get(i, ())
                    for fid in here:
                        if fid[0] == "V":
                            emit_filler(fid)
                    if diag:
                        nc.vector.tensor_mul(
                            ex[:, 0:512], ex[:, 0:512], masks[st % 4][:]
                        )
                    for j, tb in enumerate(tbs):
                        nc.tensor.matmul(
                            av[tb][0 : D + 1, :],
                            VP[st][:, lh, :],
                            ex[:, j * 512 : (j + 1) * 512],
                            start=(st == 0),
                            stop=(st == 4 * tb + 3),
                        )
                        if st == 4 * tb + 3:
                            stg = stgp.tile([128, 512], BF16, name="stg")
                            nc.vector.tensor_copy(stg[0:D, :], av[tb][0:D, :])
                            nc.sync.dma_start(
                                OT[p][hs, tb * 512 : (tb + 1) * 512],
                                stg[0:D, :],
                            )
                            stgd = stgp.tile([128, 512], F32R, name="stgd")
                            nc.vector.tensor_copy(
                                stgd[D : D + 1, :], av[tb][D : D + 1, :]
                            )
                            nc.sync.dma_start(
                                denom[lh : lh + 1, tb * 512 : (tb + 1) * 512],
                                stgd[D : D + 1, :],
                            )
                    for fid in here:
                        if fid[0] != "V":
                            emit_filler(fid)

                # normalize: OT[p] *= 1/denom broadcast over 64 partitions/head
                nc.vector.reciprocal(
                    denom[0:HL, :].bitcast(F32), denom[0:HL, :].bitcast(F32)
                )
                denb = atp.tile([128, T], BF16, name="denb")
                nc.vector.tensor_copy(denb[0:HL, :], denom[0:HL, :])
                with tc.tile_pool(name="rcpp", bufs=1) as rcpp:
                    for p in range(4):
                        rcp = rcpp.tile([128, T], BF16, name="rcp")
                        nc.sync.dma_start(
                            rcp[:], _bcast_rows(denb[2 * p : 2 * p + 2, :], 64, T)
                        )
                        nc.vector.tensor_mul(OT[p][:], OT[p][:], rcp[:])

        # ---------------- Output projection ----------------
        with tc.tile_pool(name="ysb", bufs=4) as ysb, tc.tile_pool(
            name="pps", bufs=4, space="PSUM"
        ) as pps:
            for ts in range(16):
                for cb in range(2):
                    ps = pps.tile([128, 512], F32, name="yp")
                    for p in range(4):
                        nc.tensor.matmul(
                            ps[:],
                            OT[p][:, ts * 128 : (ts + 1) * 128],
                            wpt[p][:, cb * 512 : (cb + 1) * 512],
                            start=(p == 0),
                            stop=(p == 3),
                        )
                    yt = ysb.tile([128, 512], F32, name="ys")
                    nc.vector.tensor_copy(yt[:], ps[:])
                    nc.sync.dma_start(
                        y_d[ts * 128 : (ts + 1) * 128, cb * 512 : (cb + 1) * 512],
                        yt[:],
                    )


def _build():
    nc = bacc.Bacc("TRN2", target_bir_lowering=False, debug=False)
    xT_d = nc.dram_tensor("xT", [C, T], BF16, kind="ExternalInput")
    wq_d = nc.dram_tensor("wq", [C, 512], BF16, kind="ExternalInput")
    wk_d = nc.dram_tensor("wk", [C, 512], BF16, kind="ExternalInput")
    wv_d = nc.dram_tensor("wv", [C, 512], BF16, kind="ExternalInput")
    wp_d = nc.dram_tensor("wp", [512, C], BF16, kind="ExternalInput")
    y_d = nc.dram_tensor("y", [T, C], F32, kind="ExternalOutput")
    with tile.TileContext(nc) as tc:
        _emit(nc, tc, xT_d, wq_d, wk_d, wv_d, wp_d, y_d)
    nc.compile()
    return nc


_PROG = None
LAST_EXEC_NS = None
TRACE = False
PROFILE_HOST = False


def _get_prog():
    global _PROG
    if _PROG is None:
        _PROG = _build()
    return _PROG


def _get_run(nc):
    # NOTE: must rebuild the jitted executable per call — re-executing a cached
    # loaded NEFF gives NaNs (device semaphore state is only reset on load).
    import jax
    from concourse.bass2jax import (
        Mesh,
        PartitionSpec,
        _bass_exec_p,
        install_neuronx_cc_hook,
        partition_id_tensor,
        shard_map,
    )

    install_neuronx_cc_hook()
    assert nc.dbg_addr is None

    partition_name = nc.partition_id_tensor.name if nc.partition_id_tensor else None
    in_names = []
    out_names = []
    out_avals = []
    out_shapes = []
    for alloc in nc.m.functions[0].allocations:
        if not isinstance(alloc, mybir.MemoryLocationSet):
            continue
        name = alloc.memorylocations[0].name
        if alloc.kind == "ExternalInput":
            if name != partition_name:
                in_names.append(name)
        elif alloc.kind == "ExternalOutput":
            out_names.append(name)
            shape = tuple(alloc.tensor_shape)
            dtype = mybir.dt.np(alloc.dtype)
            out_avals.append(jax.core.ShapedArray(shape, dtype))
            out_shapes.append((shape, dtype))
    n_params = len(in_names)
    n_outs = len(out_names)
    all_names = in_names + out_names + ([partition_name] if partition_name else [])
    donate = tuple(range(n_params, n_params + n_outs))

    def _body(*args):
        operands = list(args)
        if partition_name is not None:
            operands.append(partition_id_tensor())
        outs = _bass_exec_p.bind(
            *operands,
            out_avals=tuple(out_avals),
            in_names=tuple(all_names),
            out_names=tuple(out_names),
            lowering_input_output_aliases=(),
            sim_require_finite=True,
            sim_require_nnan=True,
            nc=nc,
        )
        return tuple(outs)

    devices = jax.devices()[:8]
    mesh = Mesh(np.asarray(devices), ("core",))
    in_specs = (PartitionSpec("core"),) * (n_params + n_outs)
    out_specs = (PartitionSpec("core"),) * n_outs
    sharded = jax.jit(
        shard_map(
            _body, mesh=mesh, in_specs=in_specs, out_specs=out_specs, check_rep=False
        ),
        donate_argnums=donate,
        keep_unused=True,
    )

    def run(in_maps):
        import time

        tA = time.time()
        concat_in = [
            np.concatenate([m[name] for m in in_maps], axis=0) for name in in_names
        ]
        concat_zeros = [
            np.zeros((8 * shape[0], *shape[1:]), dt) for (shape, dt) in out_shapes
        ]
        tB = time.time()
        out_arrs = sharded(*concat_in, *concat_zeros)
        jax.block_until_ready(out_arrs)
        tC = time.time()
        host = [np.asarray(a) for a in out_arrs]
        tD = time.time()
        if PROFILE_HOST:
            print(
                f"[kernel] concat {tB - tA:.3f}s  dispatch+exec {tC - tB:.3f}s  "
                f"fetch {tD - tC:.3f}s",
                flush=True,
            )
        return [
            {
                name: host[i].reshape(8, *out_shapes[i][0])[c]
                for i, name in enumerate(out_names)
            }
            for c in range(8)
        ]

    return run


def kernel(x, Wq, Wk, Wv, Wp, bp):
    global LAST_EXEC_NS
    import ml_dtypes

    BF = ml_dtypes.bfloat16
    nc = _get_prog()
    x = np.asarray(x, np.float32)
    in_maps = []
    for core in range(8):
        b, g = divmod(core, 2)
        hsl = slice(g * 8, (g + 1) * 8)
        in_maps.append(
            {
                "xT": np.ascontiguousarray(x[b].T).astype(BF),
                "wq": np.ascontiguousarray(
                    np.asarray(Wq, np.float32)[hsl].transpose(1, 0, 2).reshape(C, 512)
                ).astype(BF),
                "wk": np.ascontiguousarray(
                    np.asarray(Wk, np.float32)[hsl].transpose(1, 0, 2).reshape(C, 512)
                ).astype(BF),
                "wv": np.ascontiguousarray(
                    np.asarray(Wv, np.float32)[hsl].transpose(1, 0, 2).reshape(C, 512)
                ).astype(BF),
                "wp": np.ascontiguousarray(
                    np.asarray(Wp, np.float32)[g * 512 : (g + 1) * 512, :]
                ).astype(BF),
            }
        )
    outs = _get_run(nc)(in_maps)
    y = np.empty((B, T, C), np.float32)
    bpf = np.asarray(bp, np.float32)
    for b in range(B):
        y[b] = outs[2 * b]["y"] + outs[2 * b + 1]["y"] + bpf[None, :]
    return y
